# revision 40
# baseline (speedup 1.0000x reference)
"""AttentionDownSample Trainium2 kernel (8 NeuronCores, data-parallel over batch).

Reference computation (per batch element b):
  pooled = AvgPool2d(2)(fm)                        # [C, h, w]
  Q      = Wq @ pooled / sqrt(32)                  # [32, h, w]
  K_s    = Wk @ fm_s          (s = 2x2 window pos) # [32, h, w] x4
  logits = sum_r Q * K_s                           # [h, w, 4]
  attn   = softmax(logits, axis=-1)
  out    = sum_s fm_s * attn_s                     # [C, h, w]

Kernel strategy (per core, one batch element):
  * Qrep[32s+r, p] = Q[r, p]  via 4 PSUM-accumulated matmuls with weights
    WqT replicated x4 along free dim (folds the avg-pool into the PE).
  * Kstack[32s+r, p] = K_s[r, p] via 4 col-tiled matmuls (tile_position).
  * Mstack = Qrep * Kstack (ACT copies Qrep to SBUF bf16 first: the DVE
    can read at most one PSUM operand); logits via block-ones reduce
    matmuls packed as [4j+s, pos] for the tile's 4 chunks so the softmax
    ops run on 16 partitions at once.
  * Softmax: the Z-reduce matmul uses a block-diagonal ones weight
    [16,16] so Z comes out already replicated across the 4 window rows
    (no separate broadcast matmul); Z is ACT-copied to SBUF and inverted
    with the fast-approx DVE reciprocal (SBUF-only, partition base 0).
  * attn row broadcast over channels via one-hot-row selector matmuls;
    Y_s = fm_s * attn_bcast with the attn read directly from PSUM (DVE)
    for two window positions and via an ACT bf16 copy + GpSimd mul for
    the other two; U = sum_s Y_s via identity-weight PSUM-accumulating
    matmuls (the adds ride the TensorEngine instead of DVE).
  * Three-stage software pipeline: the casting fm DMA for tile t issues
    two iterations ahead of use (so the SW-DGE dispatch is not stuck
    behind GpSimd compute), and phase1+softmax of tile t-1 is emitted
    alongside phase3 of tile t-2 so the TensorEngine never drains (keeps
    the PE p-state at max clock) and the per-tile softmax chain hides.
All constant weight/selector matrices are precomputed on the host and passed
as extra DRAM parameters.
"""

import numpy as np
from contextlib import ExitStack

import concourse.bass as bass
import concourse.bacc as bacc_mod
import concourse.tile as tile
from concourse import mybir
from concourse.bass_utils import run_bass_kernel_spmd

F32 = mybir.dt.float32
BF16 = mybir.dt.bfloat16
AF = mybir.ActivationFunctionType

# problem dims (hardcoded; spec: fm [8,128,256,256], Wq/Wk [32,128])
B, C, H, W = 8, 128, 256, 256
PH, PW = H // 2, W // 2          # pooled 128 x 128
R = 32                           # reduce dim
QSCALE = 1.0 / (4.0 * np.sqrt(32.0))   # folds avgpool 1/4 and 1/sqrt(32)

RROWS = 32                       # raw rows per outer tile
CH = 512                         # positions per chunk (1 PSUM bank fp32)
NPACK = (RROWS // 2) * PW // CH  # chunks packed per tile (4)
# per-chunk window order: s=3 rides ACT-copy + GpSimd (longer chain,
# issued first); the rest multiply attn straight from PSUM on DVE.
S_ORDER = (3, 0, 1, 2)
S_ENGINE = {3: "gps", 0: "dve", 1: "dve", 2: "dve"}


def host_consts(Wq: np.ndarray, Wk: np.ndarray) -> dict:
    """Constant matrices computed host-side and DMA'd in once."""
    wqrep = np.tile(Wq.T.astype(np.float32) * QSCALE, (1, 4))        # [C, 128]
    wkT = np.ascontiguousarray(Wk.T.astype(np.float32))              # [C, 32]
    i128 = np.eye(C, dtype=np.float32)                               # [C, C]
    # bones packed [C, NPACK * 4*NPACK]: block j is a [C, 4*NPACK] matrix
    # whose col 4j+s has ones at rows 32s..32s+32 (zeros elsewhere, so each
    # chunk's matmul writes the full packed-logits tile).
    np4 = 4 * NPACK
    bones = np.zeros((C, NPACK * np4), dtype=np.float32)
    for j in range(NPACK):
        for s in range(4):
            bones[32 * s : 32 * s + 32, np4 * j + 4 * j + s] = 1.0
    # zsel [4*NPACK, 4*NPACK]: block-diagonal 4x4 ones — the Z matmul
    # directly produces Z replicated over each chunk's 4 window rows.
    zsel = np.zeros((4 * NPACK, 4 * NPACK), dtype=np.float32)
    for j in range(NPACK):
        zsel[4 * j : 4 * j + 4, 4 * j : 4 * j + 4] = 1.0
    # selw [4*NPACK, 4*NPACK * C]: block q ([*, C]) has row q all-ones
    selw = np.zeros((4 * NPACK, 4 * NPACK * C), dtype=np.float32)
    for q in range(4 * NPACK):
        selw[q, C * q : C * (q + 1)] = 1.0
    import ml_dtypes

    bf = {
        "wqrep": wqrep, "wkt": wkT, "i128": i128, "bones": bones,
        "zsel": zsel, "selw": selw,
    }
    return {k: v.astype(ml_dtypes.bfloat16) for k, v in bf.items()}


def build_nc(h_rows: int = H) -> bass.Bass:
    """Build the SPMD single-core program. h_rows < H shrinks the image
    height (test/sim only)."""
    assert h_rows % RROWS == 0
    ntiles = h_rows // RROWS
    prows_t = RROWS // 2                      # pooled rows per tile (16)
    npos_t = prows_t * PW                     # pooled positions per tile (2048)
    assert NPACK == npos_t // CH
    crows = CH // PW                          # pooled rows per chunk (4)
    NP4 = 4 * NPACK

    nc = bacc_mod.Bacc(
        "TRN2", target_bir_lowering=False, debug=False, num_devices=B
    )
    fm = nc.declare_dram_parameter("fm", [C, h_rows, W], F32, isOutput=False)
    cwqrep = nc.declare_dram_parameter("wqrep", [C, C], BF16, isOutput=False)
    cwkt = nc.declare_dram_parameter("wkt", [C, R], BF16, isOutput=False)
    ci128 = nc.declare_dram_parameter("i128", [C, C], BF16, isOutput=False)
    cbones = nc.declare_dram_parameter("bones", [C, NPACK * NP4], BF16, isOutput=False)
    czsel = nc.declare_dram_parameter("zsel", [NP4, NP4], BF16, isOutput=False)
    cselw = nc.declare_dram_parameter("selw", [NP4, NP4 * C], BF16, isOutput=False)
    out = nc.declare_dram_parameter("out", [C, h_rows // 2, PW], F32, isOutput=True)

    mm = nc.tensor.matmul

    with ExitStack() as ctx:
        tc = ctx.enter_context(tile.TileContext(nc))
        const = ctx.enter_context(tc.tile_pool(name="const", bufs=1))

        # ---- constants (DMA'd from host) -------------------------------
        wqrep = const.tile([C, C], BF16, tag="wqrep")
        nc.sync.dma_start(wqrep[:], cwqrep[:, :])
        wkT = const.tile([C, R], BF16, tag="wkT")
        nc.sync.dma_start(wkT[:], cwkt[:, :])
        i128 = const.tile([C, C], BF16, tag="i128")
        nc.sync.dma_start(i128[:], ci128[:, :])
        bones = const.tile([C, NPACK * NP4], BF16, tag="bones")
        nc.sync.dma_start(bones[:], cbones[:, :])
        zsel = const.tile([NP4, NP4], BF16, tag="zsel")
        nc.sync.dma_start(zsel[:], czsel[:, :])
        selw = const.tile([NP4, NP4 * C], BF16, tag="selw")
        nc.sync.dma_start(selw[:], cselw[:, :])

        # ---- pools -----------------------------------------------------
        fmp = ctx.enter_context(tc.tile_pool(name="fmp", bufs=6))
        qrs = ctx.enter_context(tc.tile_pool(name="qrs", bufs=3))
        mst = ctx.enter_context(tc.tile_pool(name="mst", bufs=3))
        esb = ctx.enter_context(tc.tile_pool(name="esb", bufs=3))
        zcp = ctx.enter_context(tc.tile_pool(name="zcp", bufs=2))
        rzp = ctx.enter_context(tc.tile_pool(name="rzp", bufs=2))
        atn = ctx.enter_context(tc.tile_pool(name="atn", bufs=3))
        ecp = ctx.enter_context(tc.tile_pool(name="ecp", bufs=4))
        yp = ctx.enter_context(tc.tile_pool(name="yp", bufs=8))
        outp = ctx.enter_context(tc.tile_pool(name="outp", bufs=2))

        pq = ctx.enter_context(tc.tile_pool(name="pq", bufs=2, space="PSUM"))
        pk = ctx.enter_context(tc.tile_pool(name="pk", bufs=2, space="PSUM"))
        plg = ctx.enter_context(tc.tile_pool(name="plg", bufs=1, space="PSUM"))
        peb = ctx.enter_context(tc.tile_pool(name="peb", bufs=2, space="PSUM"))
        pu = ctx.enter_context(tc.tile_pool(name="pu", bufs=1, space="PSUM"))

        def grid_view(fm_t):
            # grid view: [c, i(pooled row), di, j(pooled col), dj]
            return fm_t[:].rearrange(
                "c (i a j b) -> c i a j b", a=2, b=2, j=PW
            )

        def fview(grid, s, j):
            di, dj = s >> 1, s & 1
            return grid[:, crows * j : crows * (j + 1), di, :, dj]

        def load_fm(t):
            """Issue the casting DMA for tile t (2 iterations ahead of use,
            so the SW-DGE dispatch isn't stuck behind GpSimd compute).
            Split in half-tiles so downstream chunks unblock sooner."""
            fm_t = fmp.tile([C, RROWS * W], BF16, tag="fm")
            half = RROWS * W // 2
            rh = RROWS // 2
            for p in range(2):
                nc.gpsimd.dma_start(
                    fm_t[:, p * half : (p + 1) * half],
                    fm[
                        :, RROWS * t + p * rh : RROWS * t + (p + 1) * rh, :
                    ].rearrange("c h w -> c (h w)"),
                )
            return grid_view(fm_t)

        def phase1(t, grid):
            """Compute packed logits + softmax -> attn for tile t."""
            lg_ps = plg.tile([NP4, CH], F32, tag="lg")
            lg = lg_ps[:]

            for j in range(NPACK):
                qrep_ps = pq.tile([C, CH], F32, tag="pq")
                for s in range(4):
                    mm(
                        qrep_ps[:], wqrep[:], fview(grid, s, j),
                        start=(s == 0), stop=(s == 3),
                    )
                kst_ps = pk.tile([C, CH], F32, tag="pk")
                for s in range(4):
                    mm(
                        kst_ps[32 * s : 32 * s + 32, :], wkT[:], fview(grid, s, j),
                        start=True, stop=True, tile_position=(0, 32 * s),
                        skip_group_check=True,
                    )
                qrep_sb = qrs.tile([C, CH], BF16, tag="qr")
                nc.scalar.copy(qrep_sb[:], qrep_ps[:])
                m_sb = mst.tile([C, CH], BF16, tag="ms")
                nc.vector.tensor_mul(m_sb[:], qrep_sb[:], kst_ps[:])
                mm(
                    lg, bones[:, NP4 * j : NP4 * (j + 1)], m_sb[:],
                    start=(j == 0), stop=(j == NPACK - 1), skip_group_check=True,
                )

            e_sb = esb.tile([NP4, CH], BF16, tag="e")
            nc.scalar.activation(e_sb[:], lg, AF.Exp)
            # Z-reduce reuses the logits bank in place (exp already consumed it)
            mm(lg, zsel[:], e_sb[:], start=True, stop=True, skip_group_check=True)
            z_sb = zcp.tile([NP4, CH], F32, tag="zc")
            nc.scalar.copy(z_sb[:], lg)
            rz_sb = rzp.tile([NP4, CH], F32, tag="rz")
            nc.vector.reciprocal_approx_fast(rz_sb[:], z_sb[:])
            at_sb = atn.tile([NP4, CH], BF16, tag="at")
            nc.vector.tensor_mul(at_sb[:], e_sb[:], rz_sb[:])
            return at_sb

        def phase3(t, grid, at_sb):
            """Broadcast attn, window-weighted sum, write out tile t."""
            out_sb = outp.tile([C, npos_t], F32, tag="out")
            for j in range(NPACK):
                u_ps = pu.tile([C, CH], F32, tag="pu")
                ys = {}
                for s in S_ORDER:
                    q = 4 * j + s
                    e_ps = peb.tile([C, CH], F32, tag="eb")
                    mm(
                        e_ps[:], selw[:, C * q : C * (q + 1)], at_sb[:],
                        start=True, stop=True,
                    )
                    y = yp.tile([C, CH], BF16, tag="y")
                    yv = y[:].rearrange("c (i j) -> c i j", j=PW)
                    ev = e_ps[:].rearrange("c (i j) -> c i j", j=PW)
                    if S_ENGINE[s] == "dve":
                        nc.vector.tensor_mul(yv, fview(grid, s, j), ev)
                    else:
                        e_cp = ecp.tile([C, CH], BF16, tag="ec")
                        nc.scalar.copy(e_cp[:], e_ps[:])
                        nc.gpsimd.tensor_mul(
                            yv, fview(grid, s, j),
                            e_cp[:].rearrange("c (i j) -> c i j", j=PW),
                        )
                    ys[s] = y
                for k, s in enumerate(S_ORDER):
                    mm(
                        u_ps[:], i128[:], ys[s][:],
                        start=(k == 0), stop=(k == 3),
                    )
                nc.scalar.copy(out_sb[:, CH * j : CH * (j + 1)], u_ps[:])

            nc.sync.dma_start(
                out[:, prows_t * t : prows_t * (t + 1), :].rearrange(
                    "c h w -> c (h w)"
                ),
                out_sb[:],
            )

        # ---- main loop: three-stage skewed pipeline --------------------
        # iteration t: DMA tile t | logits+softmax tile t-1 | output tile t-2
        grids: dict[int, object] = {}
        attns: dict[int, object] = {}
        for t in range(ntiles + 2):
            if t < ntiles:
                grids[t] = load_fm(t)
            if 0 <= t - 1 < ntiles:
                attns[t - 1] = phase1(t - 1, grids[t - 1])
            if 0 <= t - 2 < ntiles:
                phase3(t - 2, grids[t - 2], attns.pop(t - 2))

    nc.compile()
    return nc


_CACHE: dict = {}


def _get_nc(h_rows: int = H) -> bass.Bass:
    if h_rows not in _CACHE:
        _CACHE[h_rows] = build_nc(h_rows)
    return _CACHE[h_rows]


def kernel(fm: np.ndarray, Wq: np.ndarray, Wk: np.ndarray, **run_kwargs) -> np.ndarray:
    assert fm.shape == (B, C, H, W), fm.shape
    nc = _get_nc(H)
    consts = host_consts(Wq, Wk)
    in_maps = [
        {"fm": np.ascontiguousarray(fm[b], dtype=np.float32), **consts}
        for b in range(B)
    ]
    res = run_bass_kernel_spmd(nc, in_maps, core_ids=list(range(B)), **run_kwargs)
    out = np.stack([res.results[b]["out"] for b in range(B)], axis=0)
    kernel.last_result = res
    return out


kernel.last_result = None
